# revision 11
# baseline (speedup 1.0000x reference)
"""BiLevelRoutingAttention Trainium2 kernel.

Sharding: data-parallel over (T*B)=8 cores; core = b*4 + t.
Host: windowize + transpose + region-routing top-k (0.005% of FLOPs).
Device per core (one (t,b) shard, 64 windows x 128 tokens x 256 ch):
  stage 1: qkv projection in fp32 (spike thresholds are chaotic
    below ~1e-3 qkv error, so f16 here fails); LIF spike bits. q is produced directly
    in transposed [channel, token] layout (W as the stationary operand)
    so no per-window transposes are needed; k/v bits land interleaved
    per window as [k 256 | v 256 | ones 8] in fp8e4 (bits are exact).
  stage 2 per window: gathered kv/ksum contraction as fp8 DoubleRow
    matmuls (2 k-tiles = 2 gathered windows per matmul, 2x PE rate,
    exact integer arithmetic), masked block-diagonal kv+ksum copy,
    linear attention numerator+denominator in one f16 matmul pair,
    fused reciprocal scale, f16 transpose + f16 output projection
    (f16 proj adds only ~3e-4 L2), output DMA batched 4 windows.
The top-k window indices (which depend only on batch b) are baked into
the program; cores select their variant via tc.If(partition_id).
"""

import numpy as np

# problem constants (hardcoded per contract)
T, B, Lt, Lh, Lw, C = 4, 2, 8, 32, 32, 256
WT, WH, WW = 4, 4, 4
NW = WT * WH * WW              # 64 windows
PT, PH, PW = Lt // WT, Lh // WH, Lw // WW
WS = PT * PH * PW              # 128 tokens per window
H, HD = 8, C // 8
TOPK = 4
NTOK = NW * WS                 # 8192 tokens per (t,b) shard
N_CORES = 8
WREC = 528                     # kv_bits record: k256|v256|ones8|pad8 (16B-aligned
                               # stride required by fp8 DoubleRow ldweights)

last_results = None            # stashed for test harness
last_nc = None
last_in_maps = None


def _windowize(x):
    xw = x.reshape(T, B, WT, PT, WH, PH, WW, PW, C)
    xw = xw.transpose(0, 1, 2, 4, 6, 3, 5, 7, 8).reshape(T, B, NW, WS, C)
    return xw


def _unwindowize(ow):
    o = ow.reshape(T, B, WT, WH, WW, PT, PH, PW, C)
    o = o.transpose(0, 1, 2, 5, 3, 6, 4, 7, 8).reshape(T, B, Lt, Lh, Lw, C)
    return o


def _routing_idx(xw32):
    """Mimic reference routing in fp32: region scores -> top-4 window idx."""
    region = xw32.sum(0).mean(2)                           # [B,NW,C]
    scores = np.einsum('bic,bjc->bij', region, region) * np.float32(HD ** -0.5)
    # jax.lax.top_k tie-break = lowest index first; stable argsort matches
    idx = np.argsort(-scores, axis=-1, kind='stable')[:, :, :TOPK]
    return idx                                             # [B,NW,TOPK]


def _build_program(idx_by_b, profile_single=False):
    import concourse.bass as bass
    import concourse.mybir as mybir
    import concourse.tile as tile
    from concourse import bacc
    from concourse.masks import make_identity

    f32 = mybir.dt.float32
    f16 = mybir.dt.float16
    fp8 = mybir.dt.float8e4
    DR = mybir.MatmulPerfMode.DoubleRow

    nc = bacc.Bacc("TRN2", target_bir_lowering=False, debug=False,
                   num_devices=N_CORES)

    xwT = nc.dram_tensor("xwT", [C, NTOK], f32, kind="ExternalInput").ap()
    wq = nc.dram_tensor("wq", [C, 3 * C], f32, kind="ExternalInput").ap()
    bq = nc.dram_tensor("bq", [3 * C], f32, kind="ExternalInput").ap()
    wp = nc.dram_tensor("wp", [C, C], f16, kind="ExternalInput").ap()
    bp = nc.dram_tensor("bp", [C], f32, kind="ExternalInput").ap()
    masks = nc.dram_tensor("masks", [128, 528], f32, kind="ExternalInput").ap()
    out_d = nc.dram_tensor("out", [NTOK, C], f32, kind="ExternalOutput").ap()

    with tile.TileContext(nc) as tc:
        with (
            tc.tile_pool(name="const", bufs=1) as const_pool,
            tc.tile_pool(name="bits", bufs=1) as bits_pool,
            tc.tile_pool(name="xt", bufs=3) as xt_pool,
            tc.tile_pool(name="work", bufs=3) as work_pool,
            tc.tile_pool(name="ob", bufs=2) as ob_pool,
            tc.tile_pool(name="tpsum", bufs=1, space="PSUM") as tpsum,
        ):
            # ---- resident constants ----
            wq_sb = const_pool.tile([128, 2 * 768], f32, tag="wq")
            for kc in range(2):
                nc.sync.dma_start(wq_sb[:, kc * 768:(kc + 1) * 768],
                                  wq[kc * 128:(kc + 1) * 128, :])
            wp_sb = const_pool.tile([128, 2 * 256], f16, tag="wp")
            for kc in range(2):
                nc.sync.dma_start(wp_sb[:, kc * 256:(kc + 1) * 256],
                                  wp[kc * 128:(kc + 1) * 128, :])
            ident_h = const_pool.tile([128, 128], f16, tag="idh")
            make_identity(nc, ident_h)

            ones_row = const_pool.tile([1, 128], f32, tag="ones")
            nc.vector.memset(ones_row, 1.0)
            bq_row = const_pool.tile([1, 768], f32, tag="bqr")
            nc.sync.dma_start(bq_row, bq[None, :])
            bp_row = const_pool.tile([1, 256], f32, tag="bpr")
            nc.sync.dma_start(bp_row, bp[None, :])
            mask_sb = const_pool.tile([128, 528], f32, tag="masks")
            nc.sync.dma_start(mask_sb, masks)

            # thresholds: spike(x) fires iff qkv + b >= 2
            thr_kv = const_pool.tile([128, 512], f32, tag="thrkv")
            thr_q = const_pool.tile([128, 2], f32, tag="thrq")
            bq_qT = const_pool.tile([128, 2], f32, tag="bqqt")
            nc.sync.dma_start(bq_qT, bq[0:256].rearrange("(c p) -> p c", p=128))
            nc.vector.tensor_scalar(out=thr_q, in0=bq_qT,
                                    scalar1=-1.0, scalar2=2.0,
                                    op0=mybir.AluOpType.mult,
                                    op1=mybir.AluOpType.add)
            bp_bc = const_pool.tile([128, 256], f32, tag="bpbc")

            # ---- bit tensors (resident) ----
            # q bits transposed: [qc-in-chunk, tok], chunk hq at col hq*NTOK
            qT_bits = bits_pool.tile([128, 2 * NTOK], f16, tag="qb")
            # k/v bits per window: k 0:256 | v 256:512 | ones 512:520
            kv_bits = bits_pool.tile([128, NW * WREC], fp8, tag="kvb")
            kv_r = kv_bits.rearrange("p (w d) -> p w d", d=WREC)
            nc.vector.memset(kv_r[:, :, 512:520], 1.0)

            # ---- stage 1: qkv projection + LIF bits ----
            with tc.tile_pool(name="s1_ps", bufs=3, space="PSUM") as s1_psum:
                # broadcast rows via ones-column matmul
                bc_ps = s1_psum.tile([128, 512], f32, tag="s1")
                nc.tensor.matmul(bc_ps, ones_row, bq_row[:, 256:768],
                                 start=True, stop=True)
                nc.vector.tensor_scalar(out=thr_kv, in0=bc_ps,
                                        scalar1=-1.0, scalar2=2.0,
                                        op0=mybir.AluOpType.mult,
                                        op1=mybir.AluOpType.add)
                bc_ps2 = s1_psum.tile([128, 512], f32, tag="s1")
                nc.tensor.matmul(bc_ps2[:, 0:256], ones_row, bp_row,
                                 start=True, stop=True)
                nc.scalar.copy(bp_bc, bc_ps2[:, 0:256])

                for bi in range(8):                        # 1024-token blocks
                    xt0 = xt_pool.tile([128, 1024], f32, tag="xt")
                    xt1 = xt_pool.tile([128, 1024], f32, tag="xt")
                    nc.sync.dma_start(xt0, xwT[0:128, bi * 1024:(bi + 1) * 1024])
                    nc.sync.dma_start(xt1, xwT[128:256, bi * 1024:(bi + 1) * 1024])
                    # q part: [qc, tok] = wq_q.T @ xT, 512-token sub-blocks
                    for sb in range(2):
                        tcol = bi * 1024 + sb * 512
                        for hq in range(2):
                            ps = s1_psum.tile([128, 512], f32, tag="s1")
                            nc.tensor.matmul(
                                ps, wq_sb[:, hq * 128:(hq + 1) * 128],
                                xt0[:, sb * 512:(sb + 1) * 512],
                                start=True, stop=False)
                            nc.tensor.matmul(
                                ps, wq_sb[:, 768 + hq * 128:768 + (hq + 1) * 128],
                                xt1[:, sb * 512:(sb + 1) * 512],
                                start=False, stop=True)
                            nc.vector.tensor_tensor(
                                out=qT_bits[:, hq * NTOK + tcol:
                                            hq * NTOK + tcol + 512],
                                in0=ps,
                                in1=thr_q[:, hq:hq + 1].to_broadcast([128, 512]),
                                op=mybir.AluOpType.is_ge)
                    # k/v part: [tok, c] per window
                    for wi in range(8):
                        n = bi * 8 + wi
                        ps = s1_psum.tile([128, 512], f32, tag="s1")
                        nc.tensor.matmul(
                            ps, xt0[:, wi * 128:(wi + 1) * 128],
                            wq_sb[:, 256:768], start=True, stop=False)
                        nc.tensor.matmul(
                            ps, xt1[:, wi * 128:(wi + 1) * 128],
                            wq_sb[:, 768 + 256:768 + 768],
                            start=False, stop=True)
                        nc.vector.tensor_tensor(
                            out=kv_r[:, n, 0:512], in0=ps, in1=thr_kv,
                            op=mybir.AluOpType.is_ge)

            # ---- stage 2: routed attention + projection ----
            def attention_stage(idx):
                with (
                    tc.tile_pool(name="kv_ps", bufs=3, space="PSUM") as kv_psum,
                    tc.tile_pool(name="at_ps", bufs=2, space="PSUM") as at_psum,
                    tc.tile_pool(name="pj_ps", bufs=2, space="PSUM") as pj_psum,
                ):
                    ob = None
                    for n in range(NW):
                        js = sorted(int(j) for j in idx[n])
                        pairs = [(js[0], js[1]), (js[2], js[3])]
                        kvs = work_pool.tile([128, 528], f16, tag="kvs")
                        for hf in range(2):
                            kvp = kv_psum.tile([128, 264], f32, tag="kv")
                            for pi, (ja, jb) in enumerate(pairs):
                                st = jb - ja
                                lhsT = kv_r[:, ja:jb + 1:st,
                                            hf * 128:(hf + 1) * 128]
                                rhs = kv_r[:, ja:jb + 1:st, 256:520]
                                nc.tensor.matmul(kvp, lhsT, rhs,
                                                 start=(pi == 0),
                                                 stop=(pi == 1),
                                                 perf_mode=DR)
                            nc.vector.tensor_tensor(
                                out=kvs[:, hf * 264:(hf + 1) * 264],
                                in0=kvp,
                                in1=mask_sb[:, hf * 264:(hf + 1) * 264],
                                op=mybir.AluOpType.mult)
                        # numerator + per-head D in one K=128 pair
                        ap_ = at_psum.tile([128, 264], f32, tag="at")
                        for hf in range(2):
                            nc.tensor.matmul(
                                ap_,
                                qT_bits[:, hf * NTOK + n * 128:
                                        hf * NTOK + (n + 1) * 128],
                                kvs[:, hf * 264:(hf + 1) * 264],
                                start=(hf == 0), stop=(hf == 1))
                        dr = work_pool.tile([128, 8], f32, tag="dr")
                        nc.vector.tensor_scalar_add(dr, ap_[:, 256:264], 1e-6)
                        nc.vector.reciprocal(dr, dr)
                        at = work_pool.tile([128, 256], f16, tag="attn")
                        nc.vector.tensor_tensor(
                            out=at.rearrange("p (h e) -> p h e", e=32),
                            in0=ap_[:, 0:256].rearrange("p (h e) -> p h e", e=32),
                            in1=dr.rearrange("p (h u) -> p h u", u=1)
                                 .to_broadcast([128, 8, 32]),
                            op=mybir.AluOpType.mult)
                        aT = work_pool.tile([128, 256], f16, tag="aT")
                        tp = tpsum.tile([128, 256], f16, tag="tr")
                        for kd in range(2):
                            nc.tensor.transpose(
                                tp[:, kd * 128:(kd + 1) * 128],
                                at[:, kd * 128:(kd + 1) * 128], ident_h)
                        nc.scalar.copy(aT, tp)
                        pp = pj_psum.tile([128, 256], f32, tag="pj")
                        nc.tensor.matmul(pp, aT[:, 0:128], wp_sb[:, 0:256],
                                         start=True, stop=False)
                        nc.tensor.matmul(pp, aT[:, 128:256], wp_sb[:, 256:512],
                                         start=False, stop=True)
                        w4 = n % 4
                        if w4 == 0:
                            ob = ob_pool.tile([128, 4 * 256], f32, tag="ob")
                        nc.vector.tensor_tensor(
                            out=ob[:, w4 * 256:(w4 + 1) * 256],
                            in0=pp, in1=bp_bc, op=mybir.AluOpType.add)
                        if w4 == 3:
                            n0 = n - 3
                            dst = out_d[n0 * 128:(n0 + 4) * 128, :] \
                                .rearrange("(w p) c -> p w c", p=128)
                            nc.sync.dma_start(
                                dst, ob.rearrange("p (w c) -> p w c", c=256))

            if profile_single:
                attention_stage(idx_by_b[0])
            else:
                pid = nc.partition_id()
                with tc.If(pid <= 3) as cmp:
                    attention_stage(idx_by_b[0])
                with cmp.Else():
                    attention_stage(idx_by_b[1])

    nc.compile()
    return nc


def kernel(x, W_qkv, b_qkv, W_proj, b_proj):
    global last_results, last_nc, last_in_maps
    from concourse import bass_utils

    x = np.asarray(x, dtype=np.float32)
    xw = _windowize(x)                                     # [T,B,NW,WS,C]
    idx = _routing_idx(xw)                                 # [B,NW,TOPK]

    nc = _build_program(idx)

    mask = np.zeros((128, 528), np.float32)
    for hf in range(2):
        for cr in range(128):
            h = hf * 4 + cr // 32                  # global head of row cr
            mask[cr, hf * 264 + h * 32:hf * 264 + (h + 1) * 32] = 1.0
            mask[cr, hf * 264 + 256 + h] = 1.0

    in_maps = []
    for core in range(N_CORES):
        b, t = divmod(core, T)
        xwT_c = np.ascontiguousarray(
            xw[t, b].reshape(NTOK, C).T)                   # [C, NTOK] f32
        in_maps.append({
            "xwT": xwT_c,
            "masks": mask,
            "wq": np.asarray(W_qkv, np.float32),
            "bq": np.asarray(b_qkv, np.float32),
            "wp": np.asarray(W_proj, np.float16),
            "bp": np.asarray(b_proj, np.float32),
        })

    res = bass_utils.run_bass_kernel_spmd(
        nc, in_maps, core_ids=list(range(N_CORES)), trace=False)
    last_results = res
    last_nc, last_in_maps = nc, in_maps

    ow = np.empty((T, B, NW, WS, C), np.float32)
    for core in range(N_CORES):
        b, t = divmod(core, T)
        ow[t, b] = res.results[core]["out"].reshape(NW, WS, C)
    return _unwindowize(ow)


# revision 14
# speedup vs baseline: 1.4108x; 1.4108x over previous
"""BiLevelRoutingAttention Trainium2 kernel.

Sharding: data-parallel over (T*B)=8 cores; core = b*4 + t.
Host: windowize + transpose + region-routing top-k (0.005% of FLOPs).
Device per core (one (t,b) shard, 64 windows x 128 tokens x 256 ch):
  stage 1: qkv projection in fp32 (spike thresholds are chaotic
    below ~1e-3 qkv error, so f16 here fails); LIF spike bits. q is produced directly
    in transposed [channel, token] layout (W as the stationary operand)
    so no per-window transposes are needed; k/v bits land interleaved
    per window as [k 256 | v 256 | ones 8] in fp8e4 (bits are exact).
  stage 2 per window: gathered kv/ksum contraction as fp8 DoubleRow
    matmuls (2 k-tiles = 2 gathered windows per matmul, 2x PE rate,
    exact integer arithmetic), masked block-diagonal kv+ksum copy,
    linear attention numerator+denominator in one f16 matmul pair,
    fused reciprocal scale, f16 transpose + f16 output projection
    (f16 proj adds only ~3e-4 L2), output DMA batched 4 windows.
The top-k window indices (which depend only on batch b) are baked into
the program; cores select their variant via tc.If(partition_id).
"""

import numpy as np

# problem constants (hardcoded per contract)
T, B, Lt, Lh, Lw, C = 4, 2, 8, 32, 32, 256
WT, WH, WW = 4, 4, 4
NW = WT * WH * WW              # 64 windows
PT, PH, PW = Lt // WT, Lh // WH, Lw // WW
WS = PT * PH * PW              # 128 tokens per window
H, HD = 8, C // 8
TOPK = 4
NTOK = NW * WS                 # 8192 tokens per (t,b) shard
N_CORES = 8
WREC = 528                     # kv_bits record: k256|v256|ones8|pad8 (16B-aligned
                               # stride required by fp8 DoubleRow ldweights)

last_results = None            # stashed for test harness
last_nc = None
last_in_maps = None


def _windowize(x):
    xw = x.reshape(T, B, WT, PT, WH, PH, WW, PW, C)
    xw = xw.transpose(0, 1, 2, 4, 6, 3, 5, 7, 8).reshape(T, B, NW, WS, C)
    return xw


def _unwindowize(ow):
    o = ow.reshape(T, B, WT, WH, WW, PT, PH, PW, C)
    o = o.transpose(0, 1, 2, 5, 3, 6, 4, 7, 8).reshape(T, B, Lt, Lh, Lw, C)
    return o


def _routing_idx(xw32):
    """Mimic reference routing in fp32: region scores -> top-4 window idx."""
    region = xw32.sum(0).mean(2)                           # [B,NW,C]
    scores = np.einsum('bic,bjc->bij', region, region) * np.float32(HD ** -0.5)
    # jax.lax.top_k tie-break = lowest index first; stable argsort matches
    idx = np.argsort(-scores, axis=-1, kind='stable')[:, :, :TOPK]
    return idx                                             # [B,NW,TOPK]


def _build_program(idx_by_b, profile_single=False):
    import concourse.bass as bass
    import concourse.mybir as mybir
    import concourse.tile as tile
    from concourse import bacc
    from concourse.masks import make_identity

    f32 = mybir.dt.float32
    f16 = mybir.dt.float16
    fp8 = mybir.dt.float8e4
    DR = mybir.MatmulPerfMode.DoubleRow

    nc = bacc.Bacc("TRN2", target_bir_lowering=False, debug=False,
                   num_devices=N_CORES)

    xwT = nc.dram_tensor("xwT", [C, NTOK], f32, kind="ExternalInput").ap()
    wq = nc.dram_tensor("wq", [C, 3 * C], f32, kind="ExternalInput").ap()
    bq = nc.dram_tensor("bq", [3 * C], f32, kind="ExternalInput").ap()
    wp = nc.dram_tensor("wp", [C, C], f16, kind="ExternalInput").ap()
    bp = nc.dram_tensor("bp", [C], f32, kind="ExternalInput").ap()
    masks = nc.dram_tensor("masks", [128, 528], f32, kind="ExternalInput").ap()
    out_d = nc.dram_tensor("out", [NTOK, C], f32, kind="ExternalOutput").ap()

    with tile.TileContext(nc) as tc:
        with (
            tc.tile_pool(name="const", bufs=1) as const_pool,
            tc.tile_pool(name="bits", bufs=1) as bits_pool,
            tc.tile_pool(name="xt", bufs=3) as xt_pool,
            tc.tile_pool(name="work", bufs=3) as work_pool,
            tc.tile_pool(name="ob", bufs=2) as ob_pool,
            tc.tile_pool(name="tpsum", bufs=1, space="PSUM") as tpsum,
        ):
            # ---- resident constants ----
            wq_sb = const_pool.tile([128, 2 * 768], f32, tag="wq")
            for kc in range(2):
                nc.sync.dma_start(wq_sb[:, kc * 768:(kc + 1) * 768],
                                  wq[kc * 128:(kc + 1) * 128, :])
            wp_sb = const_pool.tile([128, 2 * 256], f16, tag="wp")
            for kc in range(2):
                nc.sync.dma_start(wp_sb[:, kc * 256:(kc + 1) * 256],
                                  wp[kc * 128:(kc + 1) * 128, :])
            ident_h = const_pool.tile([128, 128], f16, tag="idh")
            make_identity(nc, ident_h)

            ones_row = const_pool.tile([1, 128], f32, tag="ones")
            nc.vector.memset(ones_row, 1.0)
            bq_row = const_pool.tile([1, 768], f32, tag="bqr")
            nc.sync.dma_start(bq_row, bq[None, :])
            bp_row = const_pool.tile([1, 256], f32, tag="bpr")
            nc.sync.dma_start(bp_row, bp[None, :])
            mask_sb = const_pool.tile([128, 528], f32, tag="masks")
            nc.sync.dma_start(mask_sb, masks)

            # thresholds: spike(x) fires iff qkv + b >= 2
            thr_kv = const_pool.tile([128, 512], f32, tag="thrkv")
            thr_q = const_pool.tile([128, 2], f32, tag="thrq")
            bq_qT = const_pool.tile([128, 2], f32, tag="bqqt")
            nc.sync.dma_start(bq_qT, bq[0:256].rearrange("(c p) -> p c", p=128))
            nc.vector.tensor_scalar(out=thr_q, in0=bq_qT,
                                    scalar1=-1.0, scalar2=2.0,
                                    op0=mybir.AluOpType.mult,
                                    op1=mybir.AluOpType.add)
            bp_bc2 = const_pool.tile([128, 512], f32, tag="bpbc")

            # ---- bit tensors (resident) ----
            # q bits transposed: [qc-in-chunk, tok], chunk hq at col hq*NTOK
            qT_bits = bits_pool.tile([128, 2 * NTOK], f16, tag="qb")
            # k/v bits per window: k 0:256 | v 256:512 | ones 512:520
            kv_bits = bits_pool.tile([128, NW * WREC], fp8, tag="kvb")
            kv_r = kv_bits.rearrange("p (w d) -> p w d", d=WREC)
            nc.vector.memset(kv_r[:, :, 512:520], 1.0)

            # ---- stage 1: qkv projection + LIF bits ----
            with tc.tile_pool(name="s1_ps", bufs=3, space="PSUM") as s1_psum:
                # broadcast rows via ones-column matmul
                bc_ps = s1_psum.tile([128, 512], f32, tag="s1")
                nc.tensor.matmul(bc_ps, ones_row, bq_row[:, 256:768],
                                 start=True, stop=True)
                nc.vector.tensor_scalar(out=thr_kv, in0=bc_ps,
                                        scalar1=-1.0, scalar2=2.0,
                                        op0=mybir.AluOpType.mult,
                                        op1=mybir.AluOpType.add)
                bc_ps2 = s1_psum.tile([128, 512], f32, tag="s1")
                nc.tensor.matmul(bc_ps2[:, 0:256], ones_row, bp_row,
                                 start=True, stop=True)
                nc.tensor.matmul(bc_ps2[:, 256:512], ones_row, bp_row,
                                 start=True, stop=True)
                nc.scalar.copy(bp_bc2, bc_ps2)

                for bi in range(8):                        # 1024-token blocks
                    xt0 = xt_pool.tile([128, 1024], f32, tag="xt")
                    xt1 = xt_pool.tile([128, 1024], f32, tag="xt")
                    nc.sync.dma_start(xt0, xwT[0:128, bi * 1024:(bi + 1) * 1024])
                    nc.sync.dma_start(xt1, xwT[128:256, bi * 1024:(bi + 1) * 1024])
                    # q part: [qc, tok] = wq_q.T @ xT, 512-token sub-blocks
                    for sb in range(2):
                        tcol = bi * 1024 + sb * 512
                        for hq in range(2):
                            ps = s1_psum.tile([128, 512], f32, tag="s1")
                            nc.tensor.matmul(
                                ps, wq_sb[:, hq * 128:(hq + 1) * 128],
                                xt0[:, sb * 512:(sb + 1) * 512],
                                start=True, stop=False)
                            nc.tensor.matmul(
                                ps, wq_sb[:, 768 + hq * 128:768 + (hq + 1) * 128],
                                xt1[:, sb * 512:(sb + 1) * 512],
                                start=False, stop=True)
                            nc.vector.tensor_tensor(
                                out=qT_bits[:, hq * NTOK + tcol:
                                            hq * NTOK + tcol + 512],
                                in0=ps,
                                in1=thr_q[:, hq:hq + 1].to_broadcast([128, 512]),
                                op=mybir.AluOpType.is_ge)
                    # k/v part: [tok, c] per window
                    for wi in range(8):
                        n = bi * 8 + wi
                        ps = s1_psum.tile([128, 512], f32, tag="s1")
                        nc.tensor.matmul(
                            ps, xt0[:, wi * 128:(wi + 1) * 128],
                            wq_sb[:, 256:768], start=True, stop=False)
                        nc.tensor.matmul(
                            ps, xt1[:, wi * 128:(wi + 1) * 128],
                            wq_sb[:, 768 + 256:768 + 768],
                            start=False, stop=True)
                        nc.vector.tensor_tensor(
                            out=kv_r[:, n, 0:512], in0=ps, in1=thr_kv,
                            op=mybir.AluOpType.is_ge)

            # ---- stage 2: routed attention + projection ----
            def attention_stage(idx):
                with (
                    tc.tile_pool(name="kv_ps", bufs=3, space="PSUM") as kv_psum,
                    tc.tile_pool(name="at_ps", bufs=2, space="PSUM") as at_psum,
                    tc.tile_pool(name="pj_ps", bufs=2, space="PSUM") as pj_psum,
                ):
                    ob = None
                    for n in range(NW):
                        js = sorted(int(j) for j in idx[n])
                        pairs = [(js[0], js[1]), (js[2], js[3])]
                        kvs = work_pool.tile([128, 528], f16, tag="kvs")
                        for hf in range(2):
                            kvp = kv_psum.tile([128, 264], f32, tag="kv")
                            for pi, (ja, jb) in enumerate(pairs):
                                st = jb - ja
                                lhsT = kv_r[:, ja:jb + 1:st,
                                            hf * 128:(hf + 1) * 128]
                                rhs = kv_r[:, ja:jb + 1:st, 256:520]
                                nc.tensor.matmul(kvp, lhsT, rhs,
                                                 start=(pi == 0),
                                                 stop=(pi == 1),
                                                 perf_mode=DR)
                            nc.vector.tensor_tensor(
                                out=kvs[:, hf * 264:(hf + 1) * 264],
                                in0=kvp,
                                in1=mask_sb[:, hf * 264:(hf + 1) * 264],
                                op=mybir.AluOpType.mult)
                        # numerator + per-head D in one K=128 pair
                        ap_ = at_psum.tile([128, 264], f32, tag="at")
                        for hf in range(2):
                            nc.tensor.matmul(
                                ap_,
                                qT_bits[:, hf * NTOK + n * 128:
                                        hf * NTOK + (n + 1) * 128],
                                kvs[:, hf * 264:(hf + 1) * 264],
                                start=(hf == 0), stop=(hf == 1))
                        dr = work_pool.tile([128, 8], f32, tag="dr")
                        nc.vector.tensor_scalar_add(dr, ap_[:, 256:264], 1e-6)
                        nc.vector.reciprocal(dr, dr)
                        at = work_pool.tile([128, 256], f16, tag="attn")
                        nc.vector.tensor_tensor(
                            out=at.rearrange("p (h e) -> p h e", e=32),
                            in0=ap_[:, 0:256].rearrange("p (h e) -> p h e", e=32),
                            in1=dr.rearrange("p (h u) -> p h u", u=1)
                                 .to_broadcast([128, 8, 32]),
                            op=mybir.AluOpType.mult)
                        aT = work_pool.tile([128, 256], f16, tag="aT")
                        tp = tpsum.tile([128, 256], f16, tag="tr")
                        for kd in range(2):
                            nc.tensor.transpose(
                                tp[:, kd * 128:(kd + 1) * 128],
                                at[:, kd * 128:(kd + 1) * 128], ident_h)
                        nc.scalar.copy(aT, tp)
                        w4 = n % 4
                        if w4 % 2 == 0:
                            pp = pj_psum.tile([128, 512], f32, tag="pj")
                        pc = (w4 % 2) * 256
                        nc.tensor.matmul(pp[:, pc:pc + 256], aT[:, 0:128],
                                         wp_sb[:, 0:256],
                                         start=True, stop=False)
                        nc.tensor.matmul(pp[:, pc:pc + 256], aT[:, 128:256],
                                         wp_sb[:, 256:512],
                                         start=False, stop=True)
                        if w4 == 0:
                            ob = ob_pool.tile([128, 4 * 256], f32, tag="ob")
                        if w4 % 2 == 1:
                            nc.vector.tensor_tensor(
                                out=ob[:, (w4 - 1) * 256:(w4 + 1) * 256],
                                in0=pp, in1=bp_bc2, op=mybir.AluOpType.add)
                        if w4 == 3:
                            n0 = n - 3
                            dst = out_d[n0 * 128:(n0 + 4) * 128, :] \
                                .rearrange("(w p) c -> p w c", p=128)
                            nc.sync.dma_start(
                                dst, ob.rearrange("p (w c) -> p w c", c=256))

            if profile_single:
                attention_stage(idx_by_b[0])
            else:
                pid = nc.partition_id()
                with tc.If(pid <= 3) as cmp:
                    attention_stage(idx_by_b[0])
                with cmp.Else():
                    attention_stage(idx_by_b[1])

    nc.compile()
    return nc


def kernel(x, W_qkv, b_qkv, W_proj, b_proj):
    global last_results, last_nc, last_in_maps
    from concourse import bass_utils

    x = np.asarray(x, dtype=np.float32)
    xw = _windowize(x)                                     # [T,B,NW,WS,C]
    idx = _routing_idx(xw)                                 # [B,NW,TOPK]

    nc = _build_program(idx)

    mask = np.zeros((128, 528), np.float32)
    for hf in range(2):
        for cr in range(128):
            h = hf * 4 + cr // 32                  # global head of row cr
            mask[cr, hf * 264 + h * 32:hf * 264 + (h + 1) * 32] = 1.0
            mask[cr, hf * 264 + 256 + h] = 1.0

    in_maps = []
    for core in range(N_CORES):
        b, t = divmod(core, T)
        xwT_c = np.ascontiguousarray(
            xw[t, b].reshape(NTOK, C).T)                   # [C, NTOK] f32
        in_maps.append({
            "xwT": xwT_c,
            "masks": mask,
            "wq": np.asarray(W_qkv, np.float32),
            "bq": np.asarray(b_qkv, np.float32),
            "wp": np.asarray(W_proj, np.float16),
            "bp": np.asarray(b_proj, np.float32),
        })

    res = bass_utils.run_bass_kernel_spmd(
        nc, in_maps, core_ids=list(range(N_CORES)), trace=False)
    last_results = res
    last_nc, last_in_maps = nc, in_maps

    ow = np.empty((T, B, NW, WS, C), np.float32)
    for core in range(N_CORES):
        b, t = divmod(core, T)
        ow[t, b] = res.results[core]["out"].reshape(NW, WS, C)
    return _unwindowize(ow)
